# revision 28
# baseline (speedup 1.0000x reference)
"""Trainium2 Bass kernel for nn_NodeSelfAttention (sparse attention block).

Sharding: 8 NeuronCores; core c handles batch b=c//4 and query rows
[w*512, (w+1)*512) of that batch (w=c%4). All attention + FFN work for those
rows is local; k/v are computed per-batch (4x replicated QKV) which avoids
any cross-core communication. The host rolls each core's batch rows so its
query window is always rows 0..511 — the SPMD program is window-agnostic.

Dataflow is kept transposed on device: scores are computed as s^T[k,q], so
the softmax denominator is a PE matmul against ones, and the adjacency mask
loads contiguously (host pre-transposes it). The per-edge bias never
materializes densely: p = exp(qk/8) * (mask01 + scatter(exp(bias)-1)),
with the scatter done by GPSIMD local_scatter from host-packed per-k-row
edge lists and the bias MLP evaluated on device (ACT Gelu / DVE).
"""

import numpy as np
import ml_dtypes

import concourse.bass as bass
import concourse.tile as tile
from concourse import bacc, mybir
from concourse.bass_utils import run_bass_kernel_spmd
from concourse.masks import make_identity

bf16 = ml_dtypes.bfloat16
F32 = mybir.dt.float32
F32R = mybir.dt.float32r
BF = mybir.dt.bfloat16
I16 = mybir.dt.int16
AF = mybir.ActivationFunctionType
ALU = mybir.AluOpType

B, S, D, H, HD = 2, 2048, 512, 8, 64
DFF = 2048
W = 512           # query window per core
NC = 8
EH = 16


# ---------------------------------------------------------------- host prep

def _prep_edges(edge_index, edge_attr, adj_mask):
    """Per-core padded edge lists keyed by ROLLED dst row, q-local src index.

    Dedup keeps the LAST edge per (src, dst) (XLA scatter-set order on CPU).
    Edges at masked-out positions are dropped (mask zeroes them anyway).
    Returns (eidx[NC, S, K] int16, eatt[NC, S, K] f32, K).
    """
    src = np.asarray(edge_index[0]).astype(np.int64)
    dst = np.asarray(edge_index[1]).astype(np.int64)
    ea = np.asarray(edge_attr, np.float32)
    E = src.shape[0]
    am = np.asarray(adj_mask)

    pairs = src * S + dst
    _, last_idx = np.unique(pairs[::-1], return_index=True)
    keep_e = E - 1 - last_idx

    sel = []
    maxk = 2
    for c in range(NC):
        b, w = c // 4, c % 4
        es = keep_e[(src[keep_e] >= w * W) & (src[keep_e] < (w + 1) * W)]
        es = es[am[b, src[es], dst[es]]]
        cnt = np.bincount(dst[es], minlength=S)
        maxk = max(maxk, int(cnt.max()))
        sel.append(es)
    K = (maxk + 1) // 2 * 2

    eidx = np.full((NC, S, K), -1, np.int16)
    eatt = np.zeros((NC, S, K), np.float32)
    for c in range(NC):
        es, w = sel[c], c % 4
        d_roll = (dst[es] - w * W) % S      # rolled k-row index
        order = np.argsort(d_roll, kind="stable")
        es, d_roll = es[order], d_roll[order]
        slot = np.arange(len(es)) - np.searchsorted(d_roll, d_roll, side="left")
        eidx[c, d_roll, slot] = (src[es] - w * W).astype(np.int16)
        eatt[c, d_roll, slot] = ea[es]
    return eidx, eatt, K


# ---------------------------------------------------------------- program

def _build_program(K):
    nc = bacc.Bacc("TRN2", target_bir_lowering=False, debug=False, num_devices=NC)

    def din(name, shape, dt):
        return nc.dram_tensor(name, shape, dt, kind="ExternalInput").ap()

    xb = din("xb", [S, D], F32)        # rolled batch rows (window first)
    m01 = din("m01", [S, W], BF)       # rolled mask^T {0,1}
    eidx_d = din("eidx", [S, K], I16)
    eattf_d = din("eattf", [K * S], BF)   # slot-major flat edge attrs
    Wq = din("Wq", [D, D], F32R)
    Wk = din("Wk", [D, D], F32R)
    Wv = din("Wv", [D, D], F32R)
    Wo = din("Wo", [D, D], F32R)
    W1 = din("W1", [D, DFF], F32R)
    W2 = din("W2", [DFF, D], F32R)
    bq = din("bq", [D], F32)
    bk = din("bk", [D], F32)
    bv = din("bv", [D], F32)
    bo = din("bo", [D], F32)
    b1 = din("b1", [DFF], F32)
    b2 = din("b2", [D], F32)
    ln1g = din("ln1g", [D], F32)
    ln1b = din("ln1b", [D], F32)
    ln2g = din("ln2g", [D], F32)
    ln2b = din("ln2b", [D], F32)
    eW1 = din("eW1", [H, EH], F32)
    eb1 = din("eb1", [H, EH], F32)
    eW2 = din("eW2", [H, EH], F32)
    eb2 = din("eb2", [H], F32)

    out = nc.dram_tensor("out", [W, D], F32, kind="ExternalOutput").ap()

    import contextlib
    with tile.TileContext(nc) as tc, contextlib.ExitStack() as ctx:
        const = ctx.enter_context(tc.tile_pool(name="const", bufs=1))
        persist = ctx.enter_context(tc.tile_pool(name="persist", bufs=1))
        ps_mm = ctx.enter_context(tc.tile_pool(name="ps_mm", bufs=4, space="PSUM"))
        ps_av = ctx.enter_context(tc.tile_pool(name="ps_av", bufs=2, space="PSUM"))

        # ---------------- constants ----------------
        ident = const.tile([128, 128], F32)
        make_identity(nc, ident)
        eps = const.tile([128, 1], F32)
        nc.vector.memset(eps, 1e-5)
        ones_bf = const.tile([128, 64], BF)
        nc.vector.memset(ones_bf, 1.0)

        # early DMAs: MLP params + edge indices (small, needed first)
        eidx_sb = const.tile([128, 16, K], I16)
        nc.sync.dma_start(out=eidx_sb, in_=eidx_d.rearrange("(c p) k -> p c k", p=128))
        w1f = const.tile([128, 1], F32)
        nc.sync.dma_start(out=w1f, in_=eW1.rearrange("h j -> (h j)")
                          .rearrange("(c p) -> p c", p=128))
        b1f = const.tile([128, 1], F32)
        nc.sync.dma_start(out=b1f, in_=eb1.rearrange("h j -> (h j)")
                          .rearrange("(c p) -> p c", p=128))
        w2f = const.tile([128, 1], F32)
        nc.sync.dma_start(out=w2f, in_=eW2.rearrange("h j -> (h j)")
                          .rearrange("(c p) -> p c", p=128))

        def chunked(v, nchunk, tag):
            t = const.tile([128, nchunk], F32, tag=tag)
            nc.sync.dma_start(out=t, in_=v.rearrange("(c p) -> p c", p=128))
            return t

        ln1g_sb = chunked(ln1g, 4, "ln1g")
        ln1b_sb = chunked(ln1b, 4, "ln1b")
        ln2g_sb = chunked(ln2g, 4, "ln2g")
        ln2b_sb = chunked(ln2b, 4, "ln2b")
        bq_sb = chunked(bq, 4, "bq")
        bk_sb = chunked(bk, 4, "bk")
        b1_sb = chunked(b1, 16, "b1")
        b2_sb = chunked(b2, 4, "b2")
        bq8_sb = const.tile([128, 4], F32)
        nc.scalar.mul(out=bq8_sb, in_=bq_sb, mul=0.125)

        def replicated(v, shape, tag):
            t = const.tile([128] + shape, F32, tag=tag)
            nc.sync.dma_start(out=t, in_=v.partition_broadcast(128))
            return t

        bv_rep = replicated(bv, [512], "bv_rep")
        bo_rep = replicated(bo, [512], "bo_rep")
        eb2r = replicated(eb2, [H], "eb2r")

        # block-diagonal w2: BD[(h,j), h'] = w2[h,j] * (h == h')
        bdw2 = const.tile([128, H], F32)
        nc.vector.memset(bdw2, 1.0)
        nc.vector.tensor_scalar_mul(out=bdw2, in0=bdw2, scalar1=w2f)
        nc.gpsimd.affine_select(out=bdw2, in_=bdw2, fill=0.0, base=0,
                                compare_op=ALU.is_ge,
                                pattern=[[-EH, H]], channel_multiplier=1)
        nc.gpsimd.affine_select(out=bdw2, in_=bdw2, fill=0.0, base=EH - 1,
                                compare_op=ALU.is_ge,
                                pattern=[[EH, H]], channel_multiplier=-1)
        bdw2_bf = const.tile([128, H], BF)
        nc.vector.tensor_copy(out=bdw2_bf, in_=bdw2)

        # persistent intermediates
        oT_sb = persist.tile([128, 4, W], F32R)   # [(h%2)*64+hd, h//2, q]
        x2_sb = persist.tile([128, 4, W], F32)    # x + attn out, natural rows

        def layernorm_T(fetch, ntiles, g_sb, b_sb, outT, work):
            """LN over row tiles; fetch(i, tile) fills x tile [128, D].
            Writes transposed f32r outT [128, 4, 128*ntiles]."""
            for i in range(ntiles):
                xt = work.tile([128, D], F32, tag="ln_x")
                fetch(i, xt)
                stats = work.tile([128, 6], F32, tag="ln_st")
                nc.vector.bn_stats(out=stats, in_=xt)
                mv = work.tile([128, 2], F32, tag="ln_mv")
                nc.vector.bn_aggr(out=mv, in_=stats)
                std = work.tile([128, 1], F32, tag="ln_sd")
                nc.scalar.activation(out=std, in_=mv[:, 1:2], func=AF.Sqrt,
                                     bias=eps, scale=1.0)
                inv = work.tile([128, 1], F32, tag="ln_iv")
                nc.vector.reciprocal(out=inv, in_=std)
                nxc = work.tile([128, D], F32, tag="ln_nx")
                nc.vector.tensor_scalar(out=nxc, in0=xt, scalar1=mv[:, 0:1],
                                        scalar2=inv, op0=ALU.subtract, op1=ALU.mult)
                for dc in range(4):
                    pt = ps_mm.tile([128, 128], F32, tag="ps")
                    nc.tensor.transpose(out=pt, in_=nxc[:, dc * 128:(dc + 1) * 128],
                                        identity=ident)
                    nc.vector.tensor_scalar(out=outT[:, dc, i * 128:(i + 1) * 128],
                                            in0=pt, scalar1=g_sb[:, dc:dc + 1],
                                            scalar2=b_sb[:, dc:dc + 1],
                                            op0=ALU.mult, op1=ALU.add)

        def load_w(dram, nchunk, tag, pool):  # [d, n] -> [128, nchunk, n]
            t = pool.tile([128, nchunk, dram.shape[1]], F32R, tag=tag)
            nc.sync.dma_start(out=t, in_=dram.rearrange("(c p) n -> p c n", p=128))
            return t

        with tc.tile_pool(name="kqv", bufs=1) as kqv:
            kT_sb = kqv.tile([128, 4, S], F32R)      # [(h%2)*64+hd, h//2, k]
            qTz = kqv.tile([128, H, W], F32R)        # head h at its half, 0 else
            v_sb = kqv.tile([128, 16, H, HD], BF)    # [k%128, k//128, h, hd]


            # ---------------- edge MLP (first: no upstream deps) ----------
            # partitions = (h, j); free = padded edges (slot-major e=s*S+krow)
            EC = K * S // 128
            with tc.tile_pool(name="dworke", bufs=1) as dwe, \
                 tc.tile_pool(name="dworks", bufs=2) as dws:
                with tc.tile_pool(name="mlp", bufs=2) as mlp:
                    attr_row = mlp.tile([1, K * S], BF, tag="attr")
                    nc.sync.dma_start(out=attr_row, in_=eattf_d[None, :])
                    ones1 = mlp.tile([1, 128], BF, tag="ones1")
                    nc.vector.memset(ones1, 1.0)
                    ps_p = ps_av.tile([128, EC * H], F32, tag="po")
                    NCH = K * S // 512
                    for ch in range(NCH):
                        ps_b = ps_mm.tile([128, 512], F32, tag="ps")
                        nc.tensor.matmul(ps_b, lhsT=ones1,
                                         rhs=attr_row[:, ch * 512:(ch + 1) * 512],
                                         start=True, stop=True)
                        hdn_ch = mlp.tile([128, 512], BF, tag="hdn")
                        nc.scalar.activation(out=hdn_ch, in_=ps_b, func=AF.Gelu,
                                             bias=b1f, scale=w1f)
                        for sub in range(4):
                            ec = ch * 4 + sub
                            nc.tensor.matmul(ps_p[:, ec * H:(ec + 1) * H],
                                             lhsT=hdn_ch[:, sub * 128:(sub + 1) * 128],
                                             rhs=bdw2_bf, start=True, stop=True)
                    scv_raw = mlp.tile([128, EC, H], BF, tag="scvr")
                    nc.vector.tensor_copy(out=scv_raw.rearrange("p c h -> p (c h)"),
                                          in_=ps_p[:, 0:EC * H])
                    scv1s = []
                    for h in range(H):
                        # e-chunk ec = s*16 + c; write transposed so dst tile
                        # is [p, c, s] (contiguous per kc)
                        src = scv_raw[:, :, h].rearrange("p (s c) -> p s c", s=K)
                        pre = mlp.tile([128, 16, K], BF, tag="pre")
                        nc.vector.tensor_scalar_add(
                            out=pre.rearrange("p c s -> p s c"),
                            in0=src, scalar1=eb2r[:, h:h + 1])
                        scv = mlp.tile([128, 16 * K], BF, tag="scv")
                        nc.scalar.activation(out=scv,
                                             in_=pre.rearrange("p c s -> p (c s)"),
                                             func=AF.Exp)
                        scv1 = dwe.tile([128, 16, K], BF, tag=f"scv1_{h}")
                        nc.vector.tensor_scalar_add(
                            out=scv1.rearrange("p c k -> p (c k)"),
                            in0=scv, scalar1=-1.0)
                        scv1s.append(scv1)

                    # ------------ phase A+B: LN1, QKV (weights streamed) ----
                    with tc.tile_pool(name="nxw", bufs=1) as nxw, \
                         tc.tile_pool(name="wch", bufs=2) as wch, \
                         tc.tile_pool(name="workA", bufs=3) as workA:
                        nxT = nxw.tile([128, 4, S], F32R)
                        Wv_sb = load_w(Wv, 4, "Wv", nxw)

                        def fetch_x(i, xt):
                            nc.sync.dma_start(
                                out=xt, in_=xb[i * 128:(i + 1) * 128, :])
                        layernorm_T(fetch_x, 16, ln1g_sb, ln1b_sb, nxT, workA)

                        # kT: out [hd-chunk 128, k 512-chunks]
                        for hc in range(4):
                            wk_ch = []
                            for dc in range(4):
                                t = wch.tile([128, 128], F32R, tag=f"wk{dc}")
                                nc.sync.dma_start(
                                    out=t, in_=Wk[dc * 128:(dc + 1) * 128,
                                                  hc * 128:(hc + 1) * 128])
                                wk_ch.append(t)
                            for rc in range(4):
                                pst = ps_mm.tile([128, 512], F32, tag="ps")
                                for dc in range(4):
                                    nc.tensor.matmul(
                                        pst, lhsT=wk_ch[dc],
                                        rhs=nxT[:, dc, rc * 512:(rc + 1) * 512],
                                        start=(dc == 0), stop=(dc == 3))
                                nc.scalar.activation(
                                    out=kT_sb[:, hc, rc * 512:(rc + 1) * 512],
                                    in_=pst, func=AF.Identity,
                                    bias=bk_sb[:, hc:hc + 1], scale=1.0)
                        # qT (window rows 0..W), scaled by 1/8, zero-padded
                        # into per-head slots of qTz
                        for hc in range(4):
                            wq_ch = []
                            for dc in range(4):
                                t = wch.tile([128, 128], F32R, tag=f"wq{dc}")
                                nc.sync.dma_start(
                                    out=t, in_=Wq[dc * 128:(dc + 1) * 128,
                                                  hc * 128:(hc + 1) * 128])
                                wq_ch.append(t)
                            pst = ps_mm.tile([128, 512], F32, tag="ps")
                            for dc in range(4):
                                nc.tensor.matmul(pst, lhsT=wq_ch[dc],
                                                 rhs=nxT[:, dc, 0:W],
                                                 start=(dc == 0), stop=(dc == 3))
                            nc.scalar.activation(out=qTz[0:64, 2 * hc, :],
                                                 in_=pst[0:64, :],
                                                 func=AF.Identity,
                                                 bias=bq8_sb[0:64, hc:hc + 1],
                                                 scale=0.125)
                            nc.scalar.activation(out=qTz[64:128, 2 * hc + 1, :],
                                                 in_=pst[64:128, :],
                                                 func=AF.Identity,
                                                 bias=bq8_sb[64:128, hc:hc + 1],
                                                 scale=0.125)
                            # zero the other head's halves
                            nc.scalar.activation(out=qTz[64:128, 2 * hc, :],
                                                 in_=pst[64:128, :],
                                                 func=AF.Identity,
                                                 bias=0.0, scale=0.0)
                            nc.scalar.activation(out=qTz[0:64, 2 * hc + 1, :],
                                                 in_=pst[0:64, :],
                                                 func=AF.Identity,
                                                 bias=0.0, scale=0.0)
                        # v natural rows: out [k-rows 128, hd 512]
                        for rc in range(16):
                            pst = ps_mm.tile([128, 512], F32, tag="ps")
                            for dc in range(4):
                                nc.tensor.matmul(pst,
                                                 lhsT=nxT[:, dc, rc * 128:(rc + 1) * 128],
                                                 rhs=Wv_sb[:, dc, :],
                                                 start=(dc == 0), stop=(dc == 3))
                            nc.vector.scalar_tensor_tensor(
                                out=v_sb[:, rc, :, :].rearrange("p h d -> p (h d)"),
                                in0=pst, scalar=1.0, in1=bv_rep,
                                op0=ALU.mult, op1=ALU.add)

                # ---------------- phase C+D: per-head attention ------------
                with tc.tile_pool(name="dwork2", bufs=2) as dw2, \
                     tc.tile_pool(name="dworky", bufs=1) as dwy, \
                     tc.tile_pool(name="dworko", bufs=2) as dwo:
                    Wo_sb = load_w(Wo, 4, "Wo", dwy)
                    # m01 loaded here (2 MB): not needed before first comb
                    m01_sb = dwy.tile([128, 16, W], BF, tag="m01")
                    nc.sync.dma_start(out=m01_sb,
                                      in_=m01.rearrange("(c p) n -> p c n", p=128))

                    def emit_av(h, pT):
                        hb = (h % 2) * 64
                        po = ps_av.tile([128, W], F32, tag="po")
                        pd = ps_av.tile([128, W], F32, tag="pd")
                        for kc in range(16):
                            nc.tensor.matmul(po[hb:hb + 64, :],
                                             lhsT=v_sb[:, kc, h, :],
                                             rhs=pT[:, kc, :],
                                             start=(kc == 0), stop=(kc == 15))
                            nc.tensor.matmul(pd[hb:hb + 64, :],
                                             lhsT=ones_bf,
                                             rhs=pT[:, kc, :],
                                             start=(kc == 0), stop=(kc == 15))
                        recip = dwo.tile([128, W], F32, tag="recip")
                        nc.vector.reciprocal(out=recip[hb:hb + 64, :],
                                             in_=pd[hb:hb + 64, :])
                        nc.vector.tensor_mul(out=oT_sb[hb:hb + 64, h // 2, :],
                                             in0=po[hb:hb + 64, :],
                                             in1=recip[hb:hb + 64, :])

                    pTs = {}
                    for h in range(H):
                        pT = dw2.tile([128, 16, W], BF, tag="pT")
                        for half in range(2):
                            k0 = half * 8
                            scat = dws.tile([128, 8, W], BF, tag="scat")
                            for kk in range(8):
                                kc = k0 + kk
                                nc.gpsimd.local_scatter(
                                    out_ap=scat[:, kk, :],
                                    data_ap=scv1s[h][:, kc, :],
                                    idxs_ap=eidx_sb[:, kc, :],
                                    channels=128, num_elems=W, num_idxs=K)
                            # comb = scat + m01 (in place)
                            nc.vector.tensor_add(
                                out=scat.rearrange("p c k -> p (c k)"),
                                in0=scat.rearrange("p c k -> p (c k)"),
                                in1=m01_sb[:, k0:k0 + 8, :]
                                .rearrange("p c k -> p (c k)"))
                            for kk in range(8):
                                kc = k0 + kk
                                ps_s = ps_mm.tile([128, W], F32, tag="ps")
                                nc.tensor.matmul(ps_s,
                                                 lhsT=kT_sb[:, h // 2,
                                                            kc * 128:(kc + 1) * 128],
                                                 rhs=qTz[:, h, :],
                                                 start=True, stop=True)
                                nc.scalar.activation(out=pT[:, kc, :], in_=ps_s,
                                                     func=AF.Exp)
                            nc.vector.tensor_mul(
                                out=pT[:, k0:k0 + 8, :].rearrange("p c k -> p (c k)"),
                                in0=pT[:, k0:k0 + 8, :].rearrange("p c k -> p (c k)"),
                                in1=scat.rearrange("p c k -> p (c k)"))
                        pTs[h] = pT
                        # software pipeline: AV for the previous head
                        if h > 0:
                            emit_av(h - 1, pTs.pop(h - 1))
                    emit_av(H - 1, pTs.pop(H - 1))

                    # ------------ phase E: out proj + residual --------------
                    yT_sb = dwy.tile([128, 4, W], F32, tag="yT")
                    for dc in range(4):
                        pst = ps_mm.tile([128, 512], F32, tag="ps")
                        for oc in range(4):
                            nc.tensor.matmul(pst,
                                             lhsT=Wo_sb[:, oc, dc * 128:(dc + 1) * 128],
                                             rhs=oT_sb[:, oc, :],
                                             start=(oc == 0), stop=(oc == 3))
                        nc.scalar.activation(out=yT_sb[:, dc, :], in_=pst,
                                             func=AF.Identity, bias=0.0, scale=1.0)
                    for qb in range(4):
                        xt = dwo.tile([128, D], F32, tag="xrow")
                        nc.sync.dma_start(out=xt, in_=xb[qb * 128:(qb + 1) * 128, :])
                        for dc in range(4):
                            pt = ps_mm.tile([128, 128], F32, tag="ps")
                            nc.tensor.transpose(
                                out=pt, in_=yT_sb[:, dc, qb * 128:(qb + 1) * 128],
                                identity=ident)
                            tchunk = dwo.tile([128, 128], F32, tag="tchunk")
                            nc.vector.scalar_tensor_tensor(
                                out=tchunk, in0=pt, scalar=1.0,
                                in1=bo_rep[:, dc * 128:(dc + 1) * 128],
                                op0=ALU.mult, op1=ALU.add)
                            nc.vector.tensor_add(
                                out=x2_sb[:, qb, dc * 128:(dc + 1) * 128],
                                in0=tchunk,
                                in1=xt[:, dc * 128:(dc + 1) * 128])

        # ---------------- phase F+G: LN2 + FFN ----------------
        with tc.tile_pool(name="ffn", bufs=1) as ffn, \
             tc.tile_pool(name="workF", bufs=3) as workF:
            nx2T = ffn.tile([128, 4, W], F32R)
            h1T = ffn.tile([128, 16, W], F32R)
            h2T = ffn.tile([128, 4, W], F32)
            out_sb = ffn.tile([128, 4, D], F32)
            W1_sb = load_w(W1, 4, "W1", ffn)
            W2_sb = load_w(W2, 16, "W2", ffn)

            def fetch_x2(i, xt):
                nc.vector.tensor_copy(out=xt, in_=x2_sb[:, i, :])
            layernorm_T(fetch_x2, 4, ln2g_sb, ln2b_sb, nx2T, workF)
            # h1T = gelu(W1^T nx2 + b1)
            for fc in range(16):
                pst = ps_mm.tile([128, 512], F32, tag="ps")
                for dc in range(4):
                    nc.tensor.matmul(pst,
                                     lhsT=W1_sb[:, dc, fc * 128:(fc + 1) * 128],
                                     rhs=nx2T[:, dc, :],
                                     start=(dc == 0), stop=(dc == 3))
                nc.scalar.activation(out=h1T[:, fc, :], in_=pst, func=AF.Gelu,
                                     bias=b1_sb[:, fc:fc + 1], scale=1.0)
            # h2T = W2^T h1 + b2
            for dc in range(4):
                pst = ps_mm.tile([128, 512], F32, tag="ps")
                for fc in range(16):
                    nc.tensor.matmul(pst,
                                     lhsT=W2_sb[:, fc, dc * 128:(dc + 1) * 128],
                                     rhs=h1T[:, fc, :],
                                     start=(fc == 0), stop=(fc == 15))
                nc.scalar.activation(out=h2T[:, dc, :], in_=pst, func=AF.Identity,
                                     bias=b2_sb[:, dc:dc + 1], scale=1.0)
            # out = x2 + h2 (transpose back to natural rows)
            for dc in range(4):
                for qb in range(4):
                    pt = ps_mm.tile([128, 128], F32, tag="ps")
                    nc.tensor.transpose(out=pt,
                                        in_=h2T[:, dc, qb * 128:(qb + 1) * 128],
                                        identity=ident)
                    nc.vector.tensor_add(
                        out=out_sb[:, qb, dc * 128:(dc + 1) * 128],
                        in0=pt, in1=x2_sb[:, qb, dc * 128:(dc + 1) * 128])
            for qb in range(4):
                nc.sync.dma_start(out=out[qb * 128:(qb + 1) * 128, :],
                                  in_=out_sb[:, qb, :])

    nc.compile()
    return nc


_PROGRAM_CACHE = {}


def _get_program(K):
    if K not in _PROGRAM_CACHE:
        _PROGRAM_CACHE[K] = _build_program(K)
    return _PROGRAM_CACHE[K]


# ---------------------------------------------------------------- entry point

def kernel(**inputs):
    x = np.asarray(inputs["x"], np.float32)
    adj_mask = np.asarray(inputs["adj_mask"])
    eidx, eatt, K = _prep_edges(inputs["edge_index"], inputs["edge_attr"], adj_mask)
    nc = _get_program(K)

    shared = {}
    for name in ("Wq", "Wk", "Wv", "Wo", "W1", "W2", "bq", "bk", "bv", "bo",
                 "b1", "b2", "ln1_g", "ln1_b", "ln2_g", "ln2_b",
                 "eW1", "eb1", "eW2", "eb2"):
        shared[name.replace("_", "")] = np.asarray(inputs[name], np.float32)

    in_maps = []
    for c in range(NC):
        b, w = c // 4, c % 4
        xb = np.roll(x[b], -w * W, axis=0)
        m01 = np.roll(adj_mask[b].T, -w * W, axis=0)[:, w * W:(w + 1) * W]
        m01 = np.ascontiguousarray(m01).astype(bf16)
        im = dict(shared)
        im["xb"] = np.ascontiguousarray(xb)
        im["m01"] = m01
        im["eidx"] = eidx[c]
        im["eattf"] = np.ascontiguousarray(eatt[c].T.reshape(-1)).astype(bf16)
        in_maps.append(im)

    res = run_bass_kernel_spmd(nc, in_maps, core_ids=list(range(NC)))
    out = np.empty((B, S, D), np.float32)
    for c in range(NC):
        b, w = c // 4, c % 4
        out[b, w * W:(w + 1) * W] = res.results[c]["out"]
    return out


# revision 35
# speedup vs baseline: 15923.5684x; 15923.5684x over previous
"""Trainium2 Bass kernel for nn_NodeSelfAttention (sparse attention block).

Sharding: 8 NeuronCores; core c handles batch b=c//4 and query rows
[w*512, (w+1)*512) of that batch (w=c%4). All attention + FFN work for those
rows is local; k/v are computed per-batch (4x replicated QKV) which avoids
any cross-core communication. The host rolls each core's batch rows so its
query window is always rows 0..511 — the SPMD program is window-agnostic.

Dataflow is kept transposed on device: scores are computed as s^T[k,q], so
the softmax denominator is a PE matmul against ones, and the adjacency mask
loads contiguously (host pre-transposes it). The per-edge bias never
materializes densely: p = exp(qk/8) * (mask01 + scatter(exp(bias)-1)),
with the scatter done by GPSIMD local_scatter from host-packed per-k-row
edge lists and the bias MLP evaluated on device (ACT Gelu / DVE).
"""

import numpy as np
import ml_dtypes

import concourse.bass as bass
import concourse.tile as tile
from concourse import bacc, mybir
from concourse.bass_utils import run_bass_kernel_spmd
from concourse.masks import make_identity

bf16 = ml_dtypes.bfloat16
F32 = mybir.dt.float32
F32R = mybir.dt.float32r
BF = mybir.dt.bfloat16
I16 = mybir.dt.int16
AF = mybir.ActivationFunctionType
ALU = mybir.AluOpType

B, S, D, H, HD = 2, 2048, 512, 8, 64
DFF = 2048
W = 512           # query window per core
NC = 8
EH = 16


# ---------------------------------------------------------------- host prep

def _prep_edges(edge_index, edge_attr, adj_mask):
    """Per-core padded edge lists keyed by ROLLED dst row, q-local src index.

    Dedup keeps the LAST edge per (src, dst) (XLA scatter-set order on CPU).
    Edges at masked-out positions are dropped (mask zeroes them anyway).
    Returns (eidx[NC, S, K] int16, eatt[NC, S, K] f32, K).
    """
    src = np.asarray(edge_index[0]).astype(np.int64)
    dst = np.asarray(edge_index[1]).astype(np.int64)
    ea = np.asarray(edge_attr, np.float32)
    E = src.shape[0]
    am = np.asarray(adj_mask)

    pairs = src * S + dst
    _, last_idx = np.unique(pairs[::-1], return_index=True)
    keep_e = E - 1 - last_idx

    sel = []
    maxk = 2
    for c in range(NC):
        b, w = c // 4, c % 4
        es = keep_e[(src[keep_e] >= w * W) & (src[keep_e] < (w + 1) * W)]
        es = es[am[b, src[es], dst[es]]]
        cnt = np.bincount(dst[es], minlength=S)
        maxk = max(maxk, int(cnt.max()))
        sel.append(es)
    K = (maxk + 1) // 2 * 2

    eidx = np.full((NC, S, K), -1, np.int16)
    eatt = np.zeros((NC, S, K), np.float32)
    for c in range(NC):
        es, w = sel[c], c % 4
        d_roll = (dst[es] - w * W) % S      # rolled k-row index
        order = np.argsort(d_roll, kind="stable")
        es, d_roll = es[order], d_roll[order]
        slot = np.arange(len(es)) - np.searchsorted(d_roll, d_roll, side="left")
        eidx[c, d_roll, slot] = (src[es] - w * W).astype(np.int16)
        eatt[c, d_roll, slot] = ea[es]
    # paired scatter calls: odd 128-row blocks address the upper 512 elems
    odd = (np.arange(S) // 128) % 2 == 1
    eidx[:, odd] = np.where(eidx[:, odd] >= 0, eidx[:, odd] + W, -1).astype(np.int16)
    return eidx, eatt, K


# ---------------------------------------------------------------- program

def _build_program(K):
    nc = bacc.Bacc("TRN2", target_bir_lowering=False, debug=False, num_devices=NC)

    def din(name, shape, dt):
        return nc.dram_tensor(name, shape, dt, kind="ExternalInput").ap()

    xb = din("xb", [S, D], F32)        # rolled batch rows (window first)
    m01 = din("m01", [S, W], BF)       # rolled mask^T {0,1}
    eidx_d = din("eidx", [S, K], I16)
    eattf_d = din("eattf", [K * S], BF)   # slot-major flat edge attrs
    Wq = din("Wq", [D, D], F32R)
    Wk = din("Wk", [D, D], F32R)
    Wv = din("Wv", [D, D], F32R)
    Wo = din("Wo", [D, D], F32R)
    W1 = din("W1", [D, DFF], F32R)
    W2 = din("W2", [DFF, D], F32R)
    bq = din("bq", [D], F32)
    bk = din("bk", [D], F32)
    bv = din("bv", [D], F32)
    bo = din("bo", [D], F32)
    b1 = din("b1", [DFF], F32)
    b2 = din("b2", [D], F32)
    ln1g = din("ln1g", [D], F32)
    ln1b = din("ln1b", [D], F32)
    ln2g = din("ln2g", [D], F32)
    ln2b = din("ln2b", [D], F32)
    eW1 = din("eW1", [H, EH], F32)
    eb1 = din("eb1", [H, EH], F32)
    eW2 = din("eW2", [H, EH], F32)
    eb2 = din("eb2", [H], F32)

    out = nc.dram_tensor("out", [W, D], F32, kind="ExternalOutput").ap()

    import contextlib
    with tile.TileContext(nc) as tc, contextlib.ExitStack() as ctx:
        const = ctx.enter_context(tc.tile_pool(name="const", bufs=1))
        persist = ctx.enter_context(tc.tile_pool(name="persist", bufs=1))
        ps_mm = ctx.enter_context(tc.tile_pool(name="ps_mm", bufs=2, space="PSUM"))
        ps_s2 = ctx.enter_context(tc.tile_pool(name="ps_s2", bufs=2, space="PSUM"))
        ps_av = ctx.enter_context(tc.tile_pool(name="ps_av", bufs=1, space="PSUM"))

        # ---------------- constants ----------------
        ident = const.tile([128, 128], F32)
        make_identity(nc, ident)
        eps = const.tile([128, 1], F32)
        nc.vector.memset(eps, 1e-5)
        ones_bf = const.tile([128, 64], BF)
        nc.vector.memset(ones_bf, 1.0)

        # early DMAs: MLP params + edge indices (small, needed first)
        eidx_sb = const.tile([128, 16, K], I16)
        nc.sync.dma_start(out=eidx_sb, in_=eidx_d.rearrange("(c p) k -> p c k", p=128))
        w1f = const.tile([128, 1], F32)
        nc.sync.dma_start(out=w1f, in_=eW1.rearrange("h j -> (h j)")
                          .rearrange("(c p) -> p c", p=128))
        b1f = const.tile([128, 1], F32)
        nc.sync.dma_start(out=b1f, in_=eb1.rearrange("h j -> (h j)")
                          .rearrange("(c p) -> p c", p=128))
        w2f = const.tile([128, 1], F32)
        nc.sync.dma_start(out=w2f, in_=eW2.rearrange("h j -> (h j)")
                          .rearrange("(c p) -> p c", p=128))

        def chunked(v, nchunk, tag):
            t = const.tile([128, nchunk], F32, tag=tag)
            nc.sync.dma_start(out=t, in_=v.rearrange("(c p) -> p c", p=128))
            return t

        def replicated(v, shape, tag):
            t = const.tile([128] + shape, F32, tag=tag)
            nc.sync.dma_start(out=t, in_=v.partition_broadcast(128))
            return t

        eb2r = replicated(eb2, [H], "eb2r")
        ln1g_sb = chunked(ln1g, 4, "ln1g")
        ln1b_sb = chunked(ln1b, 4, "ln1b")
        bq_sb = chunked(bq, 4, "bq")
        bk_sb = chunked(bk, 4, "bk")
        bq8_sb = const.tile([128, 4], F32)
        nc.scalar.mul(out=bq8_sb, in_=bq_sb, mul=0.125)

        # block-diagonal w2: BD[(h,j), h'] = w2[h,j] * (h == h')
        bdw2 = const.tile([128, H], F32)
        nc.vector.memset(bdw2, 1.0)
        nc.vector.tensor_scalar_mul(out=bdw2, in0=bdw2, scalar1=w2f)
        nc.gpsimd.affine_select(out=bdw2, in_=bdw2, fill=0.0, base=0,
                                compare_op=ALU.is_ge,
                                pattern=[[-EH, H]], channel_multiplier=1)
        nc.gpsimd.affine_select(out=bdw2, in_=bdw2, fill=0.0, base=EH - 1,
                                compare_op=ALU.is_ge,
                                pattern=[[EH, H]], channel_multiplier=-1)
        bdw2_bf = const.tile([128, H], BF)
        nc.vector.tensor_copy(out=bdw2_bf, in_=bdw2)

        # persistent intermediates
        oT_sb = persist.tile([128, 4, W], F32R)   # [(h%2)*64+hd, h//2, q]
        x2_sb = persist.tile([128, 4, W], F32)    # x + attn out, natural rows

        def layernorm_T(fetch, ntiles, g_sb, b_sb, outT, work):
            """LN over row tiles; fetch(i) returns an AP [128, D] for tile i.
            Writes transposed f32r outT [128, 4, 128*ntiles]. The transpose
            fixup (gain/bias) runs on ACT to keep DVE off the critical path."""
            for i in range(ntiles):
                xt = fetch(i)
                stats = work.tile([128, 6], F32, tag="ln_st")
                nc.vector.bn_stats(out=stats, in_=xt)
                mv = work.tile([128, 2], F32, tag="ln_mv")
                nc.vector.bn_aggr(out=mv, in_=stats)
                std = work.tile([128, 1], F32, tag="ln_sd")
                nc.scalar.activation(out=std, in_=mv[:, 1:2], func=AF.Sqrt,
                                     bias=eps, scale=1.0)
                inv = work.tile([128, 1], F32, tag="ln_iv")
                nc.vector.reciprocal(out=inv, in_=std)
                nxc = work.tile([128, D], F32, tag="ln_nx")
                nc.vector.tensor_scalar(out=nxc, in0=xt, scalar1=mv[:, 0:1],
                                        scalar2=inv, op0=ALU.subtract, op1=ALU.mult)
                for dc in range(4):
                    pt = ps_mm.tile([128, 128], F32, tag="ps")
                    nc.tensor.transpose(out=pt, in_=nxc[:, dc * 128:(dc + 1) * 128],
                                        identity=ident)
                    nc.scalar.activation(out=outT[:, dc, i * 128:(i + 1) * 128],
                                         in_=pt, func=AF.Identity,
                                         bias=b_sb[:, dc:dc + 1],
                                         scale=g_sb[:, dc:dc + 1])

        def load_w(dram, nchunk, tag, pool):  # [d, n] -> [128, nchunk, n]
            t = pool.tile([128, nchunk, dram.shape[1]], F32R, tag=tag)
            nc.sync.dma_start(out=t, in_=dram.rearrange("(c p) n -> p c n", p=128))
            return t

        with tc.tile_pool(name="kqv", bufs=1) as kqv:
            kT_sb = kqv.tile([128, 4, S], F32R)      # [(h%2)*64+hd, h//2, k]
            qTz = kqv.tile([128, H, W], F32R)        # head h at its half, 0 else
            v_sb = kqv.tile([128, 16, H, HD], BF)    # [k%128, k//128, h, hd]

            EC = K * S // 128
            with tc.tile_pool(name="dworke", bufs=1) as dwe, \
                 tc.tile_pool(name="dworks", bufs=2) as dws:
                # ---------- edge MLP (first: no upstream deps) ----------
                # partitions = (h, j); free = padded edges (slot-major)
                scv1s = []
                with tc.tile_pool(name="mlp", bufs=2) as mlp:
                    attr_row = mlp.tile([1, K * S], BF, tag="attr")
                    nc.sync.dma_start(out=attr_row, in_=eattf_d[None, :])
                    ones1 = mlp.tile([1, 128], BF, tag="ones1")
                    nc.vector.memset(ones1, 1.0)
                    ps_p = ps_av.tile([128, EC * H], F32, tag="po")
                    for ch in range(K * S // 512):
                        ps_b = ps_mm.tile([128, 512], F32, tag="ps")
                        nc.tensor.matmul(ps_b, lhsT=ones1,
                                         rhs=attr_row[:, ch * 512:(ch + 1) * 512],
                                         start=True, stop=True)
                        hdn_ch = mlp.tile([128, 512], BF, tag="hdn")
                        nc.scalar.activation(out=hdn_ch, in_=ps_b, func=AF.Gelu,
                                             bias=b1f, scale=w1f)
                        for sub in range(4):
                            ec = ch * 4 + sub
                            nc.tensor.matmul(ps_p[:, ec * H:(ec + 1) * H],
                                             lhsT=hdn_ch[:, sub * 128:(sub + 1) * 128],
                                             rhs=bdw2_bf, start=True, stop=True)
                    scv_raw = mlp.tile([128, EC, H], BF, tag="scvr")
                    nc.vector.tensor_copy(out=scv_raw.rearrange("p c h -> p (c h)"),
                                          in_=ps_p[:, 0:EC * H])
                    for h in range(H):
                        # e-chunk ec = s*16 + c; write transposed so dst tile
                        # is [p, c, s] (contiguous per kc)
                        src = scv_raw[:, :, h].rearrange("p (s c) -> p s c", s=K)
                        pre = mlp.tile([128, 16, K], BF, tag="pre")
                        nc.vector.tensor_scalar_add(
                            out=pre.rearrange("p c s -> p s c"),
                            in0=src, scalar1=eb2r[:, h:h + 1])
                        scv = mlp.tile([128, 16 * K], BF, tag="scv")
                        nc.scalar.activation(out=scv,
                                             in_=pre.rearrange("p c s -> p (c s)"),
                                             func=AF.Exp)
                        scv1 = dwe.tile([128, 16, K], BF, tag=f"scv1_{h}")
                        nc.vector.tensor_scalar_add(
                            out=scv1.rearrange("p c k -> p (c k)"),
                            in0=scv, scalar1=-1.0)
                        scv1s.append(scv1)

                # ---------- phases A+B+D interleaved ----------
                with tc.tile_pool(name="nxw", bufs=1) as nxw, \
                     tc.tile_pool(name="wch", bufs=2) as wch:
                    nxT = nxw.tile([128, 4, S], F32R)
                    with tc.tile_pool(name="workA", bufs=2) as workA:
                        slabs = {}
                        def fetch_x(i):
                            g = i // 4
                            if g not in slabs:
                                t = workA.tile([128, 4, D], F32, tag="xg")
                                nc.sync.dma_start(
                                    out=t, in_=xb.rearrange(
                                        "(c p) n -> p c n", p=128)[:, g * 4:(g + 1) * 4, :])
                                slabs[g] = t
                            return slabs[g][:, i % 4, :]
                        layernorm_T(fetch_x, 16, ln1g_sb, ln1b_sb, nxT, workA)

                    def emit_kq(hc):
                        wk_ch = []
                        for dc in range(4):
                            t = wch.tile([128, 128], F32R, tag=f"wk{dc}")
                            nc.sync.dma_start(
                                out=t, in_=Wk[dc * 128:(dc + 1) * 128,
                                              hc * 128:(hc + 1) * 128])
                            wk_ch.append(t)
                        for rc in range(4):
                            pst = ps_mm.tile([128, 512], F32, tag="ps")
                            for dc in range(4):
                                nc.tensor.matmul(
                                    pst, lhsT=wk_ch[dc],
                                    rhs=nxT[:, dc, rc * 512:(rc + 1) * 512],
                                    start=(dc == 0), stop=(dc == 3))
                            nc.scalar.activation(
                                out=kT_sb[:, hc, rc * 512:(rc + 1) * 512],
                                in_=pst, func=AF.Identity,
                                bias=bk_sb[:, hc:hc + 1], scale=1.0)
                        wq_ch = []
                        for dc in range(4):
                            t = wch.tile([128, 128], F32R, tag=f"wq{dc}")
                            nc.sync.dma_start(
                                out=t, in_=Wq[dc * 128:(dc + 1) * 128,
                                              hc * 128:(hc + 1) * 128])
                            wq_ch.append(t)
                        pst = ps_mm.tile([128, 512], F32, tag="ps")
                        for dc in range(4):
                            nc.tensor.matmul(pst, lhsT=wq_ch[dc],
                                             rhs=nxT[:, dc, 0:W],
                                             start=(dc == 0), stop=(dc == 3))
                        nc.scalar.activation(out=qTz[0:64, 2 * hc, :],
                                             in_=pst[0:64, :], func=AF.Identity,
                                             bias=bq8_sb[0:64, hc:hc + 1],
                                             scale=0.125)
                        nc.scalar.activation(out=qTz[64:128, 2 * hc + 1, :],
                                             in_=pst[64:128, :], func=AF.Identity,
                                             bias=bq8_sb[64:128, hc:hc + 1],
                                             scale=0.125)
                        nc.scalar.activation(out=qTz[64:128, 2 * hc, :],
                                             in_=pst[64:128, :], func=AF.Identity,
                                             bias=0.0, scale=0.0)
                        nc.scalar.activation(out=qTz[0:64, 2 * hc + 1, :],
                                             in_=pst[0:64, :], func=AF.Identity,
                                             bias=0.0, scale=0.0)

                    emit_kq(0)
                    # ---------- phase D (head 0 first, then v, then rest) ----
                    with tc.tile_pool(name="dwork2", bufs=2) as dw2, \
                         tc.tile_pool(name="dworkm", bufs=1) as dwm, \
                         tc.tile_pool(name="dworko", bufs=1) as dwo:
                        m01_sb = dwm.tile([128, 16, W], BF, tag="m01")
                        nc.sync.dma_start(
                            out=m01_sb, in_=m01.rearrange("(c p) n -> p c n", p=128))

                        def emit_scores(h):
                            pT = dw2.tile([128, 16, W], BF, tag="pT")
                            for half in range(2):
                                k0 = half * 8
                                scat = dws.tile([128, 4, 2 * W], BF, tag="scat")
                                for kk in range(4):
                                    kc2 = k0 + kk * 2
                                    nc.gpsimd.local_scatter(
                                        out_ap=scat[:, kk, :],
                                        data_ap=scv1s[h][:, kc2:kc2 + 2, :]
                                        .rearrange("p c k -> p (c k)"),
                                        idxs_ap=eidx_sb[:, kc2:kc2 + 2, :]
                                        .rearrange("p c k -> p (c k)"),
                                        channels=128, num_elems=2 * W,
                                        num_idxs=2 * K)
                                # comb = scat + m01 (in place)
                                nc.vector.tensor_add(
                                    out=scat.rearrange("p c k -> p (c k)"),
                                    in0=scat.rearrange("p c k -> p (c k)"),
                                    in1=m01_sb[:, k0:k0 + 8, :]
                                    .rearrange("p c k -> p (c k)"))
                                for kk in range(0, 8, 2):
                                    kc = k0 + kk
                                    ps_s = ps_s2.tile([128, 2 * W], F32, tag="ps2")
                                    for j in range(2):
                                        nc.tensor.matmul(
                                            ps_s[:, j * W:(j + 1) * W],
                                            lhsT=kT_sb[:, h // 2,
                                                       (kc + j) * 128:(kc + j + 1) * 128],
                                            rhs=qTz[:, h, :],
                                            start=True, stop=True)
                                    nc.scalar.activation(
                                        out=pT[:, kc:kc + 2, :]
                                        .rearrange("p c k -> p (c k)"),
                                        in_=ps_s, func=AF.Exp)
                                nc.vector.tensor_mul(
                                    out=pT[:, k0:k0 + 8, :]
                                    .rearrange("p c k -> p (c k)"),
                                    in0=pT[:, k0:k0 + 8, :]
                                    .rearrange("p c k -> p (c k)"),
                                    in1=scat.rearrange("p c k -> p (c k)"))
                            return pT

                        def emit_av(h, pT):
                            hb = (h % 2) * 64
                            po = ps_av.tile([128, W], F32, tag="po")
                            pd = ps_av.tile([128, W], F32, tag="pd")
                            for kc in range(16):
                                nc.tensor.matmul(po[hb:hb + 64, :],
                                                 lhsT=v_sb[:, kc, h, :],
                                                 rhs=pT[:, kc, :],
                                                 start=(kc == 0), stop=(kc == 15))
                                nc.tensor.matmul(pd[hb:hb + 64, :],
                                                 lhsT=ones_bf,
                                                 rhs=pT[:, kc, :],
                                                 start=(kc == 0), stop=(kc == 15))
                            recip = dwo.tile([128, W], F32, tag="recip")
                            nc.vector.reciprocal(out=recip[hb:hb + 64, :],
                                                 in_=pd[hb:hb + 64, :])
                            nc.vector.tensor_mul(out=oT_sb[hb:hb + 64, h // 2, :],
                                                 in0=po[hb:hb + 64, :],
                                                 in1=recip[hb:hb + 64, :])

                        pTs = {0: emit_scores(0)}
                        # v natural rows (Wv freed after); emitted after head 0
                        # so the first mul isn't stuck behind v copies on DVE
                        with tc.tile_pool(name="wvp", bufs=1) as wvp:
                            bv_rep = replicated(bv, [512], "bv_rep")
                            Wv_sb = load_w(Wv, 4, "Wv", wvp)
                            for rc in range(16):
                                pst = ps_mm.tile([128, 512], F32, tag="ps")
                                for dc in range(4):
                                    nc.tensor.matmul(
                                        pst,
                                        lhsT=nxT[:, dc, rc * 128:(rc + 1) * 128],
                                        rhs=Wv_sb[:, dc, :],
                                        start=(dc == 0), stop=(dc == 3))
                                nc.vector.scalar_tensor_tensor(
                                    out=v_sb[:, rc, :, :].rearrange("p h d -> p (h d)"),
                                    in0=pst, scalar=1.0, in1=bv_rep,
                                    op0=ALU.mult, op1=ALU.add)

                        for h in range(1, H):
                            if h % 2 == 0:
                                emit_kq(h // 2)
                            pTs[h] = emit_scores(h)
                            emit_av(h - 1, pTs.pop(h - 1))
                        emit_av(H - 1, pTs.pop(H - 1))

                # ---------- phase E: out proj + residual ----------
                with tc.tile_pool(name="dworky", bufs=1) as dwy, \
                     tc.tile_pool(name="dworkye", bufs=2) as dwye:
                    bo_rep = replicated(bo, [512], "bo_rep")
                    Wo_sb = load_w(Wo, 4, "Wo", dwy)
                    yT_sb = dwy.tile([128, 4, W], F32, tag="yT")
                    for dc in range(4):
                        pst = ps_mm.tile([128, 512], F32, tag="ps")
                        for oc in range(4):
                            nc.tensor.matmul(pst,
                                             lhsT=Wo_sb[:, oc, dc * 128:(dc + 1) * 128],
                                             rhs=oT_sb[:, oc, :],
                                             start=(oc == 0), stop=(oc == 3))
                        nc.scalar.activation(out=yT_sb[:, dc, :], in_=pst,
                                             func=AF.Identity, bias=0.0, scale=1.0)
                    for qb in range(4):
                        xt = dwye.tile([128, D], F32, tag="xrow")
                        nc.sync.dma_start(out=xt, in_=xb[qb * 128:(qb + 1) * 128, :])
                        for dc in range(4):
                            pt = ps_mm.tile([128, 128], F32, tag="ps")
                            nc.tensor.transpose(
                                out=pt, in_=yT_sb[:, dc, qb * 128:(qb + 1) * 128],
                                identity=ident)
                            tchunk = dwye.tile([128, 128], F32, tag="tchunk")
                            nc.vector.scalar_tensor_tensor(
                                out=tchunk, in0=pt, scalar=1.0,
                                in1=bo_rep[:, dc * 128:(dc + 1) * 128],
                                op0=ALU.mult, op1=ALU.add)
                            nc.vector.tensor_add(
                                out=x2_sb[:, qb, dc * 128:(dc + 1) * 128],
                                in0=tchunk,
                                in1=xt[:, dc * 128:(dc + 1) * 128])

        # ---------------- phase F+G: LN2 + FFN ----------------
        with tc.tile_pool(name="ffn", bufs=1) as ffn, \
             tc.tile_pool(name="workF", bufs=3) as workF:
            nx2T = ffn.tile([128, 4, W], F32R)
            h1T = ffn.tile([128, 16, W], F32R)
            h2T = ffn.tile([128, 4, W], F32)
            out_sb = ffn.tile([128, 4, D], F32)
            ln2g_sb = chunked(ln2g, 4, "ln2g")
            ln2b_sb = chunked(ln2b, 4, "ln2b")
            b1_sb = chunked(b1, 16, "b1")
            b2_sb = chunked(b2, 4, "b2")
            W1_sb = load_w(W1, 4, "W1", ffn)
            W2_sb = load_w(W2, 16, "W2", ffn)

            layernorm_T(lambda i: x2_sb[:, i, :], 4, ln2g_sb, ln2b_sb,
                        nx2T, workF)
            # h1T = gelu(W1^T nx2 + b1)
            for fc in range(16):
                pst = ps_mm.tile([128, 512], F32, tag="ps")
                for dc in range(4):
                    nc.tensor.matmul(pst,
                                     lhsT=W1_sb[:, dc, fc * 128:(fc + 1) * 128],
                                     rhs=nx2T[:, dc, :],
                                     start=(dc == 0), stop=(dc == 3))
                nc.scalar.activation(out=h1T[:, fc, :], in_=pst, func=AF.Gelu,
                                     bias=b1_sb[:, fc:fc + 1], scale=1.0)
            # h2T = W2^T h1 + b2
            for dc in range(4):
                pst = ps_mm.tile([128, 512], F32, tag="ps")
                for fc in range(16):
                    nc.tensor.matmul(pst,
                                     lhsT=W2_sb[:, fc, dc * 128:(dc + 1) * 128],
                                     rhs=h1T[:, fc, :],
                                     start=(fc == 0), stop=(fc == 15))
                nc.scalar.activation(out=h2T[:, dc, :], in_=pst, func=AF.Identity,
                                     bias=b2_sb[:, dc:dc + 1], scale=1.0)
            # out = x2 + h2 (transpose back to natural rows)
            for dc in range(4):
                for qb in range(4):
                    pt = ps_mm.tile([128, 128], F32, tag="ps")
                    nc.tensor.transpose(out=pt,
                                        in_=h2T[:, dc, qb * 128:(qb + 1) * 128],
                                        identity=ident)
                    nc.vector.tensor_add(
                        out=out_sb[:, qb, dc * 128:(dc + 1) * 128],
                        in0=pt, in1=x2_sb[:, qb, dc * 128:(dc + 1) * 128])
            for qb in range(4):
                nc.sync.dma_start(out=out[qb * 128:(qb + 1) * 128, :],
                                  in_=out_sb[:, qb, :])

    nc.compile()
    return nc


_PROGRAM_CACHE = {}


def _get_program(K):
    if K not in _PROGRAM_CACHE:
        _PROGRAM_CACHE[K] = _build_program(K)
    return _PROGRAM_CACHE[K]


# ---------------------------------------------------------------- entry point

def kernel(**inputs):
    x = np.asarray(inputs["x"], np.float32)
    adj_mask = np.asarray(inputs["adj_mask"])
    eidx, eatt, K = _prep_edges(inputs["edge_index"], inputs["edge_attr"], adj_mask)
    nc = _get_program(K)

    shared = {}
    for name in ("Wq", "Wk", "Wv", "Wo", "W1", "W2", "bq", "bk", "bv", "bo",
                 "b1", "b2", "ln1_g", "ln1_b", "ln2_g", "ln2_b",
                 "eW1", "eb1", "eW2", "eb2"):
        shared[name.replace("_", "")] = np.asarray(inputs[name], np.float32)

    in_maps = []
    for c in range(NC):
        b, w = c // 4, c % 4
        xb = np.roll(x[b], -w * W, axis=0)
        m01 = np.roll(adj_mask[b].T, -w * W, axis=0)[:, w * W:(w + 1) * W]
        m01 = np.ascontiguousarray(m01).astype(bf16)
        im = dict(shared)
        im["xb"] = np.ascontiguousarray(xb)
        im["m01"] = m01
        im["eidx"] = eidx[c]
        im["eattf"] = np.ascontiguousarray(eatt[c].T.reshape(-1)).astype(bf16)
        in_maps.append(im)

    res = run_bass_kernel_spmd(nc, in_maps, core_ids=list(range(NC)))
    out = np.empty((B, S, D), np.float32)
    for c in range(NC):
        b, w = c // 4, c % 4
        out[b, w * W:(w + 1) * W] = res.results[c]["out"]
    return out
